# revision 6
# baseline (speedup 1.0000x reference)
"""Trainium2 Bass kernel for the non-local-block module (nn_CNL_747324309589).

Sharding: data-parallel over batch — 16 batches across 8 NeuronCores, 2 per
core, no collectives.  Per batch (dims: HIGH=2048, LOW=512, N=H*W=1152):

    theta_xT[n,c] = sum_h xh[h,n]·thwT[h,c] + thb[c]      (x_h chunks = lhsT)
    phi_xT [n,d]  = sum_l xl[l,n]·phwT[l,d] + phb[d]      (phw,phb pre-scaled by 1/512)
    g_x    [d,n]  = sum_l gwT[l,d]·xl[l,n]  + gb[d]
    attT   [d,c]  = sum_n phi_xT[n,d]·theta_xT[n,c]       (= energy^T/512)
    y      [c,n]  = sum_d attT[d,c]·g_x[d,n]
    w_y    [o,n]  = sum_c wwT[c,o]·y[c,n]                 (BN scale pre-folded into ww)
    out    [o,n]  = w_y + bnt[o] + xh[o,n]                (shift + residual in one DVE op)

All matmul operands are bf16 (same 1 row/cycle PE rate as float32r in the
cost model, half the DMA bytes and SBUF footprint), accumulating fp32 in
PSUM; the output is DMA'd out as bf16 and widened to fp32 on the host.
Weights are loaded once (not per batch).  theta accumulates k-major across 8
PSUM banks so each (thw quarter, x_h chunk) pair is consumed as it lands;
phase order A2(phi), A1(theta), A3(g) puts g between theta's drain burst and
B1.  Batch b+1's x_l / x_h prefetch DMAs issue from the otherwise idle ACT
queue inside batch b's C loop.
"""

import numpy as np

import concourse.bass as bass
import concourse.bacc as bacc
import concourse.mybir as mybir
import concourse.tile as tile
from concourse.bass import ts

B, HIGH, LOW, H, W = 16, 2048, 512, 48, 24
N = H * W            # 1152
NCORES = 8
BPC = B // NCORES    # 2 batches per core
P = 128
KH = HIGH // P       # 16
KL = LOW // P        # 4
MN = N // P          # 9
NSPLIT = 3
NW = N // NSPLIT     # 384
BN_EPS = 1e-5

F32 = mybir.dt.float32
BF16 = mybir.dt.bfloat16
ADD = mybir.AluOpType.add
AF = mybir.ActivationFunctionType


def _build_module() -> bass.Bass:
    nc = bacc.Bacc()
    x_h = nc.dram_tensor("x_h", [BPC, HIGH, N], BF16, kind="ExternalInput")
    x_l = nc.dram_tensor("x_l", [BPC, LOW, N], BF16, kind="ExternalInput")
    thw = nc.dram_tensor("thw", [P, KH, LOW], BF16, kind="ExternalInput")
    phw = nc.dram_tensor("phw", [P, KL, LOW], BF16, kind="ExternalInput")
    gw = nc.dram_tensor("gw", [P, KL, LOW], BF16, kind="ExternalInput")
    ww = nc.dram_tensor("ww", [P, KL, HIGH], BF16, kind="ExternalInput")
    thpb = nc.dram_tensor("thpb", [1, 2 * LOW], BF16, kind="ExternalInput")
    gbnt = nc.dram_tensor("gbnt", [P, KL + KH], F32, kind="ExternalInput")
    out = nc.dram_tensor("out", [BPC, HIGH, N], BF16, kind="ExternalOutput")

    with tile.TileContext(nc) as tc:
        with (
            tc.tile_pool(name="consts", bufs=1) as cpool,
            tc.tile_pool(name="xh", bufs=KH) as xhpool,
            tc.tile_pool(name="xl", bufs=1) as xlpool,
            tc.tile_pool(name="att", bufs=1) as attpool,
            tc.tile_pool(name="mid", bufs=1) as midpool,
            tc.tile_pool(name="stg", bufs=3) as stgpool,
            tc.tile_pool(name="psum", bufs=8, space="PSUM") as pspool,
        ):
            # PE warmup: the p-state ramp (0.65/1.2 GHz for the first ~3us of
            # PE activity) burns on throwaway matmuls while the first DMAs
            # land, so real matmuls start at the full 2.4 GHz clock
            wu = cpool.tile([P, 64], BF16, tag="wu")
            nc.gpsimd.memset(wu[:], 0.0)
            wps = pspool.tile([P, 512], F32, tag="ps", name="wps")
            for i in range(120):
                nc.tensor.matmul(
                    wps[:64, :64], wu[:], wu[:], start=True, stop=True
                )

            # A2's inputs go first so phi can start ASAP; phw is split into
            # k-chunks and x_l into m-chunks so the m=0 group starts on the
            # first ~220KB instead of the full 0.9MB.
            phw_sb = cpool.tile([P, KL, LOW], BF16, tag="phw")
            for k in range(KL):
                nc.sync.dma_start(phw_sb[:, k], phw[:, k])
            xl0_sb = xlpool.tile([P, KL, N], BF16, tag="xl")
            xl0_r = x_l[0].rearrange("(ko p) n -> p ko n", p=P)
            for m in range(MN):
                nc.sync.dma_start(
                    xl0_sb[:, :, ts(m, P)], xl0_r[:, :, ts(m, P)]
                )
            thpb_sb = cpool.tile([P, 2 * LOW], BF16, tag="thpb")
            nc.sync.dma_start(thpb_sb[:], thpb[:].to_broadcast((P, 2 * LOW)))
            thb_sb = thpb_sb[:, :LOW]
            phb_sb = thpb_sb[:, LOW:]
            gbnt_sb = cpool.tile([P, KL + KH], F32, tag="gbnt")
            nc.sync.dma_start(gbnt_sb[:], gbnt[:])
            gb_sb = gbnt_sb[:, :KL]
            bnt_sb = gbnt_sb[:, KL:]
            # theta weights + batch-0 x_h chunks, interleaved in k-major
            # consumption order (A1 uses thw[k], xh[k] at k-step k); issued
            # from the ACT HWDGE queue so they don't serialize behind the
            # phi-phase DMAs on SP's queue
            thw_sb = cpool.tile([P, KH, LOW], BF16, tag="thw")
            xh_t: list = [None] * KH
            for q in range(4):
                nc.scalar.dma_start(
                    thw_sb[:, ts(q, KH // 4), :], thw[:, ts(q, KH // 4), :]
                )
                for k in range(q * 4, q * 4 + 4):
                    t_ = xhpool.tile([P, N], BF16, tag="xh")
                    nc.scalar.dma_start(t_[:], x_h[0, ts(k, P), :])
                    xh_t[k] = t_
            gw_sb = cpool.tile([P, KL, LOW], BF16, tag="gw")
            nc.sync.dma_start(gw_sb[:], gw[:])
            ww_sb = cpool.tile([P, KL, HIGH], BF16, tag="ww")
            for k in range(KL):
                nc.sync.dma_start(ww_sb[:, k], ww[:, k])

            for b in range(BPC):
                if b > 0:
                    xl_sb = xl_next
                    xh_t = xh_next

                else:
                    xl_sb = xl0_sb

                # phi_xT [n, d] (phase A2)
                ph_sb = midpool.tile([P, MN, LOW], BF16, tag="ph")
                for m in range(MN):
                    ps = pspool.tile([P, 512], F32, tag="ps")
                    for k in range(KL):
                        nc.tensor.matmul(
                            ps[:],
                            xl_sb[:, k, ts(m, P)],
                            phw_sb[:, k, :],
                            start=(k == 0),
                            stop=(k == KL - 1),
                        )
                    nc.vector.tensor_tensor(ph_sb[:, m, :], ps[:], phb_sb[:], ADD)

                # theta_xT [n, c] (phase A1) — k-major over 8 PSUM banks so
                # chunk k is consumed right after it lands; m=8 runs m-major
                th_sb = midpool.tile([P, MN, LOW], BF16, tag="th")
                ps_a1 = [
                    pspool.tile([P, 512], F32, tag="ps", name=f"ps_a1_{m}")
                    for m in range(8)
                ]
                for k in range(KH):
                    for m in range(8):
                        nc.tensor.matmul(
                            ps_a1[m][:],
                            xh_t[k][:, ts(m, P)],
                            thw_sb[:, k, :],
                            start=(k == 0),
                            stop=(k == KH - 1),
                        )
                for m in range(8):
                    nc.vector.tensor_tensor(th_sb[:, m, :], ps_a1[m][:], thb_sb[:], ADD)
                ps = pspool.tile([P, 512], F32, tag="ps")
                for k in range(KH):
                    nc.tensor.matmul(
                        ps[:],
                        xh_t[k][:, ts(8, P)],
                        thw_sb[:, k, :],
                        start=(k == 0),
                        stop=(k == KH - 1),
                    )
                nc.vector.tensor_tensor(th_sb[:, 8, :], ps[:], thb_sb[:], ADD)

                # g_x [d, n] (phase A3) — sits between theta's drain burst
                # and B1 so the th drains overlap PE work
                g_sb = midpool.tile([P, KL, N], BF16, tag="g")
                for md in range(KL):
                    for nn in range(NSPLIT):
                        ps = pspool.tile([P, 512], F32, tag="ps")
                        for k in range(KL):
                            nc.tensor.matmul(
                                ps[:, :NW],
                                gw_sb[:, k, ts(md, P)],
                                xl_sb[:, k, ts(nn, NW)],
                                start=(k == 0),
                                stop=(k == KL - 1),
                            )
                        nc.scalar.activation(
                            g_sb[:, md, ts(nn, NW)],
                            ps[:, :NW],
                            AF.Identity,
                            bias=gb_sb[:, md : md + 1],
                        )

                # batch b+1 x_l prefetch: WAR on this batch's A2/A3 reads,
                # issued from the ACT queue (behind A3's drains)
                if b + 1 < BPC:
                    xl_next = xlpool.tile([P, KL, N], BF16, tag="xl")
                    xl1_r = x_l[b + 1].rearrange("(ko p) n -> p ko n", p=P)
                    for nn in range(NSPLIT):
                        nc.scalar.dma_start(
                            xl_next[:, :, ts(nn, NW)], xl1_r[:, :, ts(nn, NW)]
                        )

                # attT [d, c] = energy^T/512 (phase B1)
                att_sb = attpool.tile([P, KL, LOW], BF16, tag="att")
                for md in range(KL):
                    ps = pspool.tile([P, 512], F32, tag="ps")
                    for k in range(MN):
                        nc.tensor.matmul(
                            ps[:],
                            ph_sb[:, k, ts(md, P)],
                            th_sb[:, k, :],
                            start=(k == 0),
                            stop=(k == MN - 1),
                        )
                    nc.scalar.activation(att_sb[:, md, :], ps[:], AF.Identity)

                # y [c, n] (phase B2); y shares the theta_xT slot
                y_sb = midpool.tile([P, KL, N], BF16, tag="th")
                for mc in range(KL):
                    for nn in range(NSPLIT):
                        ps = pspool.tile([P, 512], F32, tag="ps")
                        for k in range(KL):
                            nc.tensor.matmul(
                                ps[:, :NW],
                                att_sb[:, k, ts(mc, P)],
                                g_sb[:, k, ts(nn, NW)],
                                start=(k == 0),
                                stop=(k == KL - 1),
                            )
                        nc.scalar.activation(
                            y_sb[:, mc, ts(nn, NW)], ps[:, :NW], AF.Identity
                        )

                # w_y + BN + residual (phase C); output staged per mo stripe
                # and written as one DMA; batch b+1 x_h chunk prefetch issues
                # from ACT right after chunk mo's last read
                for mo in range(KH):
                    xt = xh_t[mo]
                    stg = stgpool.tile([P, N], BF16, tag="stg")
                    last = b == BPC - 1 and mo == KH - 1
                    for nn in range(NSPLIT):
                        ps = pspool.tile([P, 512], F32, tag="ps")
                        for k in range(KL):
                            nc.tensor.matmul(
                                ps[:, :NW],
                                ww_sb[:, k, ts(mo, P)],
                                y_sb[:, k, ts(nn, NW)],
                                start=(k == 0),
                                stop=(k == KL - 1),
                            )
                        # the very last stripe drains in small pieces so the
                        # final STT+DMA after the last matmul is short
                        pieces = [(0, NW)] if not last else (
                            [(0, NW)] if nn < NSPLIT - 1 else [(0, NW // 2), (NW // 2, NW // 2)]
                        )
                        for off, w in pieces:
                            nc.vector.scalar_tensor_tensor(
                                stg[:, nn * NW + off : nn * NW + off + w],
                                ps[:, off : off + w],
                                bnt_sb[:, mo : mo + 1],
                                xt[:, nn * NW + off : nn * NW + off + w],
                                ADD,
                                ADD,
                            )
                            if last:
                                nc.sync.dma_start(
                                    out[b, ts(mo, P), nn * NW + off : nn * NW + off + w],
                                    stg[:, nn * NW + off : nn * NW + off + w],
                                )
                    if not last:
                        nc.sync.dma_start(out[b, ts(mo, P), :], stg[:])
                    if b + 1 < BPC:
                        if mo == 0:
                            xh_next = [None] * KH
                        t_ = xhpool.tile([P, N], BF16, tag="xh")
                        nc.scalar.dma_start(t_[:], x_h[b + 1, ts(mo, P), :])
                        xh_next[mo] = t_
    nc.compile()
    return nc


_CACHE: dict = {}


def _get_module() -> bass.Bass:
    if "nc" not in _CACHE:
        _CACHE["nc"] = _build_module()
    return _CACHE["nc"]


def _prep_maps(inputs: dict) -> list[dict]:
    import ml_dtypes

    BF = ml_dtypes.bfloat16
    f = lambda a: np.ascontiguousarray(np.asarray(a, dtype=np.float32))
    bf = lambda a: np.ascontiguousarray(np.asarray(a, dtype=np.float32).astype(BF))
    x_h = bf(inputs["x_h"]).reshape(B, HIGH, N)
    x_l = bf(inputs["x_l"]).reshape(B, LOW, N)
    theta_w = f(inputs["theta_w"])
    phi_w = f(inputs["phi_w"])
    g_w = f(inputs["g_w"])
    w_w = f(inputs["w_w"])

    thw_h = theta_w.T.reshape(KH, P, LOW).transpose(1, 0, 2).astype(BF)
    phw_h = (phi_w.T / np.float32(LOW)).reshape(KL, P, LOW).transpose(1, 0, 2).astype(BF)
    gw_h = g_w.T.reshape(KL, P, LOW).transpose(1, 0, 2).astype(BF)
    s = f(inputs["bn_gamma"]) / np.sqrt(f(inputs["bn_var"]) + np.float32(BN_EPS))
    # BN scale folded into the w conv weights; only the shift remains on-device
    ww_h = (w_w * s[:, None]).astype(np.float32).T.reshape(KL, P, HIGH) \
        .transpose(1, 0, 2).astype(BF)

    thpb_h = np.concatenate(
        [f(inputs["theta_b"]), f(inputs["phi_b"]) / np.float32(LOW)]
    ).reshape(1, 2 * LOW).astype(BF)
    gb_h = np.ascontiguousarray(f(inputs["g_b"]).reshape(KL, P).T)
    t = (f(inputs["w_b"]) - f(inputs["bn_mean"])) * s + f(inputs["bn_beta"])
    bnt_h = np.ascontiguousarray(t.astype(np.float32).reshape(KH, P).T)
    gbnt_h = np.ascontiguousarray(np.concatenate([gb_h, bnt_h], axis=1))

    shared = dict(
        thw=np.ascontiguousarray(thw_h),
        phw=np.ascontiguousarray(phw_h),
        gw=np.ascontiguousarray(gw_h),
        ww=np.ascontiguousarray(ww_h),
        thpb=thpb_h,
        gbnt=gbnt_h,
    )
    maps = []
    for c in range(NCORES):
        m = dict(shared)
        m["x_h"] = np.ascontiguousarray(x_h[c * BPC : (c + 1) * BPC])
        m["x_l"] = np.ascontiguousarray(x_l[c * BPC : (c + 1) * BPC])
        maps.append(m)
    return maps


def _run(inputs: dict, **kwargs):
    from concourse.bass_utils import run_bass_kernel_spmd

    nc = _get_module()
    in_maps = _prep_maps(inputs)
    res = run_bass_kernel_spmd(nc, in_maps, core_ids=list(range(NCORES)), **kwargs)
    parts = [np.asarray(r["out"], dtype=np.float32) for r in res.results]
    full = np.concatenate(parts, axis=0).reshape(B, HIGH, H, W)
    return full, res


def kernel(**inputs) -> np.ndarray:
    full, _ = _run(inputs)
    return full


# revision 9
# speedup vs baseline: 1.0147x; 1.0147x over previous
"""Trainium2 Bass kernel for the non-local-block module (nn_CNL_747324309589).

Sharding: data-parallel over batch — 16 batches across 8 NeuronCores, 2 per
core, no collectives.  Per batch (dims: HIGH=2048, LOW=512, N=H*W=1152):

    theta_xT[n,c] = sum_h xh[h,n]·thwT[h,c] + thb[c]      (x_h chunks = lhsT)
    phi_xT [n,d]  = sum_l xl[l,n]·phwT[l,d] + phb[d]      (phw,phb pre-scaled by 1/512)
    g_x    [d,n]  = sum_l gwT[l,d]·xl[l,n]  + gb[d]
    attT   [d,c]  = sum_n phi_xT[n,d]·theta_xT[n,c]       (= energy^T/512)
    y      [c,n]  = sum_d attT[d,c]·g_x[d,n]
    w_y    [o,n]  = sum_c wwT[c,o]·y[c,n]                 (BN scale pre-folded into ww)
    out    [o,n]  = w_y + bnt[o] + xh[o,n]                (shift + residual in one DVE op)

All matmul operands are bf16 (same 1 row/cycle PE rate as float32r in the
cost model, half the DMA bytes and SBUF footprint), accumulating fp32 in
PSUM; the output is DMA'd out as bf16 and widened to fp32 on the host.
Weights are loaded once (not per batch).  theta accumulates k-major across 8
PSUM banks so each (thw quarter, x_h chunk) pair is consumed as it lands;
phase order A2(phi), A1(theta), A3(g) puts g between theta's drain burst and
B1.  Batch b+1's x_l / x_h prefetch DMAs issue from the otherwise idle ACT
queue inside batch b's C loop.
"""

import numpy as np

import concourse.bass as bass
import concourse.bacc as bacc
import concourse.mybir as mybir
import concourse.tile as tile
from concourse.bass import ts

B, HIGH, LOW, H, W = 16, 2048, 512, 48, 24
N = H * W            # 1152
NCORES = 8
BPC = B // NCORES    # 2 batches per core
P = 128
KH = HIGH // P       # 16
KL = LOW // P        # 4
MN = N // P          # 9
NSPLIT = 3
NW = N // NSPLIT     # 384
BN_EPS = 1e-5

F32 = mybir.dt.float32
BF16 = mybir.dt.bfloat16
ADD = mybir.AluOpType.add
AF = mybir.ActivationFunctionType


def _build_module() -> bass.Bass:
    nc = bacc.Bacc()
    x_h = nc.dram_tensor("x_h", [BPC, HIGH, N], BF16, kind="ExternalInput")
    x_l = nc.dram_tensor("x_l", [BPC, LOW, N], BF16, kind="ExternalInput")
    thw = nc.dram_tensor("thw", [P, KH, LOW], BF16, kind="ExternalInput")
    phw = nc.dram_tensor("phw", [P, KL, LOW], BF16, kind="ExternalInput")
    gw = nc.dram_tensor("gw", [P, KL, LOW], BF16, kind="ExternalInput")
    ww = nc.dram_tensor("ww", [P, KL, HIGH], BF16, kind="ExternalInput")
    thpb = nc.dram_tensor("thpb", [1, 2 * LOW], BF16, kind="ExternalInput")
    gbnt = nc.dram_tensor("gbnt", [P, KL + KH], F32, kind="ExternalInput")
    out = nc.dram_tensor("out", [BPC, HIGH, N], BF16, kind="ExternalOutput")

    with tile.TileContext(nc) as tc:
        with (
            tc.tile_pool(name="consts", bufs=1) as cpool,
            tc.tile_pool(name="xh", bufs=KH) as xhpool,
            tc.tile_pool(name="xl", bufs=1) as xlpool,
            tc.tile_pool(name="att", bufs=1) as attpool,
            tc.tile_pool(name="mid", bufs=1) as midpool,
            tc.tile_pool(name="stg", bufs=3) as stgpool,
            tc.tile_pool(name="psum", bufs=8, space="PSUM") as pspool,
        ):
            # PE warmup: the p-state ramp (0.65/1.2 GHz for the first ~3us of
            # PE activity) burns on throwaway matmuls while the first DMAs
            # land, so real matmuls start at the full 2.4 GHz clock
            wu = cpool.tile([P, 64], BF16, tag="wu")
            nc.gpsimd.memset(wu[:], 0.0)
            wps = pspool.tile([P, 512], F32, tag="ps", name="wps")
            for i in range(135):
                nc.tensor.matmul(
                    wps[:64, :64], wu[:], wu[:], start=True, stop=True
                )

            # A2's inputs go first so phi can start ASAP.  Each dma_start
            # costs ~1.2us serialized on the issuing queue, so keep the count
            # low: one phw transfer, three x_l n-chunks.
            phw_sb = cpool.tile([P, KL, LOW], BF16, tag="phw")
            nc.sync.dma_start(phw_sb[:], phw[:])
            xl0_sb = xlpool.tile([P, KL, N], BF16, tag="xl")
            xl0_r = x_l[0].rearrange("(ko p) n -> p ko n", p=P)
            for nn in range(NSPLIT):
                nc.sync.dma_start(
                    xl0_sb[:, :, ts(nn, NW)], xl0_r[:, :, ts(nn, NW)]
                )
            thpb_sb = cpool.tile([P, 2 * LOW], BF16, tag="thpb")
            nc.sync.dma_start(thpb_sb[:], thpb[:].to_broadcast((P, 2 * LOW)))
            thb_sb = thpb_sb[:, :LOW]
            phb_sb = thpb_sb[:, LOW:]
            gbnt_sb = cpool.tile([P, KL + KH], F32, tag="gbnt")
            nc.sync.dma_start(gbnt_sb[:], gbnt[:])
            gb_sb = gbnt_sb[:, :KL]
            bnt_sb = gbnt_sb[:, KL:]
            # theta weights + batch-0 x_h chunks, interleaved in k-major
            # consumption order (A1 uses thw[k], xh[k] at k-step k)
            thw_sb = cpool.tile([P, KH, LOW], BF16, tag="thw")
            xh_t: list = [None] * KH
            for q in range(4):
                nc.sync.dma_start(
                    thw_sb[:, ts(q, KH // 4), :], thw[:, ts(q, KH // 4), :]
                )
                for k in range(q * 4, q * 4 + 4):
                    t_ = xhpool.tile([P, N], BF16, tag="xh")
                    nc.sync.dma_start(t_[:], x_h[0, ts(k, P), :])
                    xh_t[k] = t_
            gw_sb = cpool.tile([P, KL, LOW], BF16, tag="gw")
            nc.sync.dma_start(gw_sb[:], gw[:])
            ww_sb = cpool.tile([P, KL, HIGH], BF16, tag="ww")
            for k in range(KL):
                nc.sync.dma_start(ww_sb[:, k], ww[:, k])

            for b in range(BPC):
                if b > 0:
                    xl_sb = xl_next
                    xh_t = xh_next

                else:
                    xl_sb = xl0_sb

                # phi_xT [n, d] (phase A2)
                ph_sb = midpool.tile([P, MN, LOW], BF16, tag="ph")
                for m in range(MN):
                    ps = pspool.tile([P, 512], F32, tag="ps")
                    for k in range(KL):
                        nc.tensor.matmul(
                            ps[:],
                            xl_sb[:, k, ts(m, P)],
                            phw_sb[:, k, :],
                            start=(k == 0),
                            stop=(k == KL - 1),
                        )
                    nc.vector.tensor_tensor(ph_sb[:, m, :], ps[:], phb_sb[:], ADD)

                # theta_xT [n, c] (phase A1) — k-major over 8 PSUM banks so
                # chunk k is consumed right after it lands; m=8 runs m-major
                th_sb = midpool.tile([P, MN, LOW], BF16, tag="th")
                ps_a1 = [
                    pspool.tile([P, 512], F32, tag="ps", name=f"ps_a1_{m}")
                    for m in range(8)
                ]
                for k in range(KH):
                    for m in range(8):
                        nc.tensor.matmul(
                            ps_a1[m][:],
                            xh_t[k][:, ts(m, P)],
                            thw_sb[:, k, :],
                            start=(k == 0),
                            stop=(k == KH - 1),
                        )
                for m in range(8):
                    nc.vector.tensor_tensor(th_sb[:, m, :], ps_a1[m][:], thb_sb[:], ADD)
                ps = pspool.tile([P, 512], F32, tag="ps")
                for k in range(KH):
                    nc.tensor.matmul(
                        ps[:],
                        xh_t[k][:, ts(8, P)],
                        thw_sb[:, k, :],
                        start=(k == 0),
                        stop=(k == KH - 1),
                    )
                nc.vector.tensor_tensor(th_sb[:, 8, :], ps[:], thb_sb[:], ADD)

                # g_x [d, n] (phase A3) — sits between theta's drain burst
                # and B1 so the th drains overlap PE work
                g_sb = midpool.tile([P, KL, N], BF16, tag="g")
                for md in range(KL):
                    for nn in range(NSPLIT):
                        ps = pspool.tile([P, 512], F32, tag="ps")
                        for k in range(KL):
                            nc.tensor.matmul(
                                ps[:, :NW],
                                gw_sb[:, k, ts(md, P)],
                                xl_sb[:, k, ts(nn, NW)],
                                start=(k == 0),
                                stop=(k == KL - 1),
                            )
                        nc.scalar.activation(
                            g_sb[:, md, ts(nn, NW)],
                            ps[:, :NW],
                            AF.Identity,
                            bias=gb_sb[:, md : md + 1],
                        )

                # batch b+1 x_l prefetch: WAR on this batch's A2/A3 reads,
                # issued from the ACT queue (behind A3's drains)
                if b + 1 < BPC:
                    xl_next = xlpool.tile([P, KL, N], BF16, tag="xl")
                    xl1_r = x_l[b + 1].rearrange("(ko p) n -> p ko n", p=P)
                    for nn in range(NSPLIT):
                        nc.scalar.dma_start(
                            xl_next[:, :, ts(nn, NW)], xl1_r[:, :, ts(nn, NW)]
                        )

                # attT [d, c] = energy^T/512 (phase B1)
                att_sb = attpool.tile([P, KL, LOW], BF16, tag="att")
                for md in range(KL):
                    ps = pspool.tile([P, 512], F32, tag="ps")
                    for k in range(MN):
                        nc.tensor.matmul(
                            ps[:],
                            ph_sb[:, k, ts(md, P)],
                            th_sb[:, k, :],
                            start=(k == 0),
                            stop=(k == MN - 1),
                        )
                    nc.scalar.activation(att_sb[:, md, :], ps[:], AF.Identity)

                # y [c, n] (phase B2); y shares the theta_xT slot
                y_sb = midpool.tile([P, KL, N], BF16, tag="th")
                for mc in range(KL):
                    for nn in range(NSPLIT):
                        ps = pspool.tile([P, 512], F32, tag="ps")
                        for k in range(KL):
                            nc.tensor.matmul(
                                ps[:, :NW],
                                att_sb[:, k, ts(mc, P)],
                                g_sb[:, k, ts(nn, NW)],
                                start=(k == 0),
                                stop=(k == KL - 1),
                            )
                        nc.scalar.activation(
                            y_sb[:, mc, ts(nn, NW)], ps[:, :NW], AF.Identity
                        )

                # w_y + BN + residual (phase C); output staged per mo stripe
                # and written as one DMA; batch b+1 x_h chunk prefetch issues
                # from ACT right after chunk mo's last read
                for mo in range(KH):
                    xt = xh_t[mo]
                    stg = stgpool.tile([P, N], BF16, tag="stg")
                    last = b == BPC - 1 and mo == KH - 1
                    for nn in range(NSPLIT):
                        ps = pspool.tile([P, 512], F32, tag="ps")
                        for k in range(KL):
                            nc.tensor.matmul(
                                ps[:, :NW],
                                ww_sb[:, k, ts(mo, P)],
                                y_sb[:, k, ts(nn, NW)],
                                start=(k == 0),
                                stop=(k == KL - 1),
                            )
                        nc.vector.scalar_tensor_tensor(
                            stg[:, ts(nn, NW)],
                            ps[:, :NW],
                            bnt_sb[:, mo : mo + 1],
                            xt[:, ts(nn, NW)],
                            ADD,
                            ADD,
                        )
                        # the very last stripe writes out in two DMAs so the
                        # final transfer after the last matmul is small
                        if last and nn == NSPLIT - 2:
                            nc.sync.dma_start(
                                out[b, ts(mo, P), : 2 * NW], stg[:, : 2 * NW]
                            )
                        elif last and nn == NSPLIT - 1:
                            nc.sync.dma_start(
                                out[b, ts(mo, P), 2 * NW :], stg[:, 2 * NW :]
                            )
                    if not last:
                        nc.sync.dma_start(out[b, ts(mo, P), :], stg[:])
                    if b + 1 < BPC:
                        if mo == 0:
                            xh_next = [None] * KH
                        t_ = xhpool.tile([P, N], BF16, tag="xh")
                        nc.scalar.dma_start(t_[:], x_h[b + 1, ts(mo, P), :])
                        xh_next[mo] = t_
    nc.compile()
    return nc


_CACHE: dict = {}


def _get_module() -> bass.Bass:
    if "nc" not in _CACHE:
        _CACHE["nc"] = _build_module()
    return _CACHE["nc"]


def _prep_maps(inputs: dict) -> list[dict]:
    import ml_dtypes

    BF = ml_dtypes.bfloat16
    f = lambda a: np.ascontiguousarray(np.asarray(a, dtype=np.float32))
    bf = lambda a: np.ascontiguousarray(np.asarray(a, dtype=np.float32).astype(BF))
    x_h = bf(inputs["x_h"]).reshape(B, HIGH, N)
    x_l = bf(inputs["x_l"]).reshape(B, LOW, N)
    theta_w = f(inputs["theta_w"])
    phi_w = f(inputs["phi_w"])
    g_w = f(inputs["g_w"])
    w_w = f(inputs["w_w"])

    thw_h = theta_w.T.reshape(KH, P, LOW).transpose(1, 0, 2).astype(BF)
    phw_h = (phi_w.T / np.float32(LOW)).reshape(KL, P, LOW).transpose(1, 0, 2).astype(BF)
    gw_h = g_w.T.reshape(KL, P, LOW).transpose(1, 0, 2).astype(BF)
    s = f(inputs["bn_gamma"]) / np.sqrt(f(inputs["bn_var"]) + np.float32(BN_EPS))
    # BN scale folded into the w conv weights; only the shift remains on-device
    ww_h = (w_w * s[:, None]).astype(np.float32).T.reshape(KL, P, HIGH) \
        .transpose(1, 0, 2).astype(BF)

    thpb_h = np.concatenate(
        [f(inputs["theta_b"]), f(inputs["phi_b"]) / np.float32(LOW)]
    ).reshape(1, 2 * LOW).astype(BF)
    gb_h = np.ascontiguousarray(f(inputs["g_b"]).reshape(KL, P).T)
    t = (f(inputs["w_b"]) - f(inputs["bn_mean"])) * s + f(inputs["bn_beta"])
    bnt_h = np.ascontiguousarray(t.astype(np.float32).reshape(KH, P).T)
    gbnt_h = np.ascontiguousarray(np.concatenate([gb_h, bnt_h], axis=1))

    shared = dict(
        thw=np.ascontiguousarray(thw_h),
        phw=np.ascontiguousarray(phw_h),
        gw=np.ascontiguousarray(gw_h),
        ww=np.ascontiguousarray(ww_h),
        thpb=thpb_h,
        gbnt=gbnt_h,
    )
    maps = []
    for c in range(NCORES):
        m = dict(shared)
        m["x_h"] = np.ascontiguousarray(x_h[c * BPC : (c + 1) * BPC])
        m["x_l"] = np.ascontiguousarray(x_l[c * BPC : (c + 1) * BPC])
        maps.append(m)
    return maps


def _run(inputs: dict, **kwargs):
    from concourse.bass_utils import run_bass_kernel_spmd

    nc = _get_module()
    in_maps = _prep_maps(inputs)
    res = run_bass_kernel_spmd(nc, in_maps, core_ids=list(range(NCORES)), **kwargs)
    parts = [np.asarray(r["out"], dtype=np.float32) for r in res.results]
    full = np.concatenate(parts, axis=0).reshape(B, HIGH, H, W)
    return full, res


def kernel(**inputs) -> np.ndarray:
    full, _ = _run(inputs)
    return full


# revision 11
# speedup vs baseline: 1.0212x; 1.0063x over previous
"""Trainium2 Bass kernel for the non-local-block module (nn_CNL_747324309589).

Sharding: data-parallel over batch — 16 batches across 8 NeuronCores, 2 per
core, no collectives.  Per batch (dims: HIGH=2048, LOW=512, N=H*W=1152):

    theta_xT[n,c] = sum_h xh[h,n]·thwT[h,c] + thb[c]      (x_h chunks = lhsT)
    phi_xT [n,d]  = sum_l xl[l,n]·phwT[l,d] + phb[d]      (phw,phb pre-scaled by 1/512)
    g_x    [d,n]  = sum_l gwT[l,d]·xl[l,n]  + gb[d]
    attT   [d,c]  = sum_n phi_xT[n,d]·theta_xT[n,c]       (= energy^T/512)
    y      [c,n]  = sum_d attT[d,c]·g_x[d,n]
    w_y    [o,n]  = sum_c wwT[c,o]·y[c,n]                 (BN scale pre-folded into ww)
    out    [o,n]  = w_y + bnt[o] + xh[o,n]                (shift + residual in one DVE op)

All matmul operands are bf16 (same 1 row/cycle PE rate as float32r in the
cost model, half the DMA bytes and SBUF footprint), accumulating fp32 in
PSUM; the output is DMA'd out as bf16 and widened to fp32 on the host.
Weights are loaded once (not per batch).  theta accumulates k-major across 8
PSUM banks so each (thw quarter, x_h chunk) pair is consumed as it lands;
phase order A2(phi), A1(theta), A3(g) puts g between theta's drain burst and
B1.  Batch b+1's x_l / x_h prefetch DMAs issue from the otherwise idle ACT
queue inside batch b's C loop.
"""

import numpy as np

import concourse.bass as bass
import concourse.bacc as bacc
import concourse.mybir as mybir
import concourse.tile as tile
from concourse.bass import ts

B, HIGH, LOW, H, W = 16, 2048, 512, 48, 24
N = H * W            # 1152
NCORES = 8
BPC = B // NCORES    # 2 batches per core
P = 128
KH = HIGH // P       # 16
KL = LOW // P        # 4
MN = N // P          # 9
NSPLIT = 3
NW = N // NSPLIT     # 384
BN_EPS = 1e-5

F32 = mybir.dt.float32
BF16 = mybir.dt.bfloat16
ADD = mybir.AluOpType.add
AF = mybir.ActivationFunctionType


def _build_module() -> bass.Bass:
    nc = bacc.Bacc()
    x_h = nc.dram_tensor("x_h", [BPC, HIGH, N], BF16, kind="ExternalInput")
    x_l = nc.dram_tensor("x_l", [BPC, LOW, N], BF16, kind="ExternalInput")
    thw = nc.dram_tensor("thw", [P, KH, LOW], BF16, kind="ExternalInput")
    phw = nc.dram_tensor("phw", [P, KL, LOW], BF16, kind="ExternalInput")
    gw = nc.dram_tensor("gw", [P, KL, LOW], BF16, kind="ExternalInput")
    ww = nc.dram_tensor("ww", [P, KL, HIGH], BF16, kind="ExternalInput")
    thpb = nc.dram_tensor("thpb", [1, 2 * LOW], BF16, kind="ExternalInput")
    gbnt = nc.dram_tensor("gbnt", [P, KL + KH], F32, kind="ExternalInput")
    out = nc.dram_tensor("out", [BPC, HIGH, N], BF16, kind="ExternalOutput")

    with tile.TileContext(nc) as tc:
        with (
            tc.tile_pool(name="consts", bufs=1) as cpool,
            tc.tile_pool(name="xh", bufs=KH) as xhpool,
            tc.tile_pool(name="xl", bufs=1) as xlpool,
            tc.tile_pool(name="att", bufs=1) as attpool,
            tc.tile_pool(name="mid", bufs=1) as midpool,
            tc.tile_pool(name="stg", bufs=3) as stgpool,
            tc.tile_pool(name="psum", bufs=8, space="PSUM") as pspool,
        ):
            # PE warmup: the p-state ramp (0.65/1.2 GHz for the first ~3us of
            # PE activity) burns on throwaway matmuls while the first DMAs
            # land, so real matmuls start at the full 2.4 GHz clock
            wu = cpool.tile([P, 64], BF16, tag="wu")
            nc.gpsimd.memset(wu[:], 0.0)
            wps = pspool.tile([P, 512], F32, tag="ps", name="wps")
            for i in range(90):
                nc.tensor.matmul(
                    wps[:64, :64], wu[:], wu[:], start=True, stop=True
                )

            # A2's inputs go first so phi can start ASAP.  Each dma_start
            # costs ~1.2us serialized on its issuing queue, so the three
            # first-dependency transfers are spread across SP, Pool (SWDGE)
            # and ACT so their issue latencies overlap.
            phw_sb = cpool.tile([P, KL, LOW], BF16, tag="phw")
            nc.sync.dma_start(phw_sb[:, :2], phw[:, :2])
            nc.gpsimd.dma_start(phw_sb[:, 2:], phw[:, 2:])
            xl0_sb = xlpool.tile([P, KL, N], BF16, tag="xl")
            xl0_r = x_l[0].rearrange("(ko p) n -> p ko n", p=P)
            for nn in range(NSPLIT):
                nc.scalar.dma_start(
                    xl0_sb[:, :, ts(nn, NW)], xl0_r[:, :, ts(nn, NW)]
                )
            thpb_sb = cpool.tile([P, 2 * LOW], BF16, tag="thpb")
            nc.sync.dma_start(thpb_sb[:], thpb[:].to_broadcast((P, 2 * LOW)))
            thb_sb = thpb_sb[:, :LOW]
            phb_sb = thpb_sb[:, LOW:]
            gbnt_sb = cpool.tile([P, KL + KH], F32, tag="gbnt")
            nc.sync.dma_start(gbnt_sb[:], gbnt[:])
            gb_sb = gbnt_sb[:, :KL]
            bnt_sb = gbnt_sb[:, KL:]
            # theta weights + batch-0 x_h chunks, interleaved in k-major
            # consumption order (A1 uses thw[k], xh[k] at k-step k)
            thw_sb = cpool.tile([P, KH, LOW], BF16, tag="thw")
            xh_t: list = [None] * KH
            for q in range(4):
                nc.sync.dma_start(
                    thw_sb[:, ts(q, KH // 4), :], thw[:, ts(q, KH // 4), :]
                )
                for k in range(q * 4, q * 4 + 4):
                    t_ = xhpool.tile([P, N], BF16, tag="xh")
                    nc.sync.dma_start(t_[:], x_h[0, ts(k, P), :])
                    xh_t[k] = t_
            gw_sb = cpool.tile([P, KL, LOW], BF16, tag="gw")
            nc.scalar.dma_start(gw_sb[:], gw[:])
            ww_sb = cpool.tile([P, KL, HIGH], BF16, tag="ww")
            for k in range(KL):
                nc.sync.dma_start(ww_sb[:, k], ww[:, k])

            for b in range(BPC):
                if b > 0:
                    xl_sb = xl_next
                    xh_t = xh_next

                else:
                    xl_sb = xl0_sb

                # phi_xT [n, d] (phase A2)
                ph_sb = midpool.tile([P, MN, LOW], BF16, tag="ph")
                for m in range(MN):
                    ps = pspool.tile([P, 512], F32, tag="ps")
                    for k in range(KL):
                        nc.tensor.matmul(
                            ps[:],
                            xl_sb[:, k, ts(m, P)],
                            phw_sb[:, k, :],
                            start=(k == 0),
                            stop=(k == KL - 1),
                        )
                    nc.vector.tensor_tensor(ph_sb[:, m, :], ps[:], phb_sb[:], ADD)

                # theta_xT [n, c] (phase A1) — k-major over 8 PSUM banks so
                # chunk k is consumed right after it lands; m=8 runs m-major
                th_sb = midpool.tile([P, MN, LOW], BF16, tag="th")
                ps_a1 = [
                    pspool.tile([P, 512], F32, tag="ps", name=f"ps_a1_{m}")
                    for m in range(8)
                ]
                for k in range(KH):
                    for m in range(8):
                        nc.tensor.matmul(
                            ps_a1[m][:],
                            xh_t[k][:, ts(m, P)],
                            thw_sb[:, k, :],
                            start=(k == 0),
                            stop=(k == KH - 1),
                        )
                for m in range(8):
                    nc.vector.tensor_tensor(th_sb[:, m, :], ps_a1[m][:], thb_sb[:], ADD)
                ps = pspool.tile([P, 512], F32, tag="ps")
                for k in range(KH):
                    nc.tensor.matmul(
                        ps[:],
                        xh_t[k][:, ts(8, P)],
                        thw_sb[:, k, :],
                        start=(k == 0),
                        stop=(k == KH - 1),
                    )
                nc.vector.tensor_tensor(th_sb[:, 8, :], ps[:], thb_sb[:], ADD)

                # g_x [d, n] (phase A3) — sits between theta's drain burst
                # and B1 so the th drains overlap PE work
                g_sb = midpool.tile([P, KL, N], BF16, tag="g")
                for md in range(KL):
                    for nn in range(NSPLIT):
                        ps = pspool.tile([P, 512], F32, tag="ps")
                        for k in range(KL):
                            nc.tensor.matmul(
                                ps[:, :NW],
                                gw_sb[:, k, ts(md, P)],
                                xl_sb[:, k, ts(nn, NW)],
                                start=(k == 0),
                                stop=(k == KL - 1),
                            )
                        nc.scalar.activation(
                            g_sb[:, md, ts(nn, NW)],
                            ps[:, :NW],
                            AF.Identity,
                            bias=gb_sb[:, md : md + 1],
                        )

                # batch b+1 x_l prefetch: WAR on this batch's A2/A3 reads,
                # issued from the ACT queue (behind A3's drains)
                if b + 1 < BPC:
                    xl_next = xlpool.tile([P, KL, N], BF16, tag="xl")
                    xl1_r = x_l[b + 1].rearrange("(ko p) n -> p ko n", p=P)
                    for nn in range(NSPLIT):
                        nc.scalar.dma_start(
                            xl_next[:, :, ts(nn, NW)], xl1_r[:, :, ts(nn, NW)]
                        )

                # attT [d, c] = energy^T/512 (phase B1)
                att_sb = attpool.tile([P, KL, LOW], BF16, tag="att")
                for md in range(KL):
                    ps = pspool.tile([P, 512], F32, tag="ps")
                    for k in range(MN):
                        nc.tensor.matmul(
                            ps[:],
                            ph_sb[:, k, ts(md, P)],
                            th_sb[:, k, :],
                            start=(k == 0),
                            stop=(k == MN - 1),
                        )
                    nc.scalar.activation(att_sb[:, md, :], ps[:], AF.Identity)

                # y [c, n] (phase B2); y shares the theta_xT slot
                y_sb = midpool.tile([P, KL, N], BF16, tag="th")
                for mc in range(KL):
                    for nn in range(NSPLIT):
                        ps = pspool.tile([P, 512], F32, tag="ps")
                        for k in range(KL):
                            nc.tensor.matmul(
                                ps[:, :NW],
                                att_sb[:, k, ts(mc, P)],
                                g_sb[:, k, ts(nn, NW)],
                                start=(k == 0),
                                stop=(k == KL - 1),
                            )
                        nc.scalar.activation(
                            y_sb[:, mc, ts(nn, NW)], ps[:, :NW], AF.Identity
                        )

                # w_y + BN + residual (phase C); output staged per mo stripe
                # and written as one DMA; batch b+1 x_h chunk prefetch issues
                # from ACT right after chunk mo's last read
                for mo in range(KH):
                    xt = xh_t[mo]
                    stg = stgpool.tile([P, N], BF16, tag="stg")
                    last = b == BPC - 1 and mo == KH - 1
                    for nn in range(NSPLIT):
                        ps = pspool.tile([P, 512], F32, tag="ps")
                        for k in range(KL):
                            nc.tensor.matmul(
                                ps[:, :NW],
                                ww_sb[:, k, ts(mo, P)],
                                y_sb[:, k, ts(nn, NW)],
                                start=(k == 0),
                                stop=(k == KL - 1),
                            )
                        nc.vector.scalar_tensor_tensor(
                            stg[:, ts(nn, NW)],
                            ps[:, :NW],
                            bnt_sb[:, mo : mo + 1],
                            xt[:, ts(nn, NW)],
                            ADD,
                            ADD,
                        )
                        # the very last stripe writes out in two DMAs so the
                        # final transfer after the last matmul is small
                        if last and nn == NSPLIT - 2:
                            nc.sync.dma_start(
                                out[b, ts(mo, P), : 2 * NW], stg[:, : 2 * NW]
                            )
                        elif last and nn == NSPLIT - 1:
                            nc.sync.dma_start(
                                out[b, ts(mo, P), 2 * NW :], stg[:, 2 * NW :]
                            )
                    if not last:
                        nc.sync.dma_start(out[b, ts(mo, P), :], stg[:])
                    if b + 1 < BPC:
                        if mo == 0:
                            xh_next = [None] * KH
                        t_ = xhpool.tile([P, N], BF16, tag="xh")
                        nc.scalar.dma_start(t_[:], x_h[b + 1, ts(mo, P), :])
                        xh_next[mo] = t_
    nc.compile()
    return nc


_CACHE: dict = {}


def _get_module() -> bass.Bass:
    if "nc" not in _CACHE:
        _CACHE["nc"] = _build_module()
    return _CACHE["nc"]


def _prep_maps(inputs: dict) -> list[dict]:
    import ml_dtypes

    BF = ml_dtypes.bfloat16
    f = lambda a: np.ascontiguousarray(np.asarray(a, dtype=np.float32))
    bf = lambda a: np.ascontiguousarray(np.asarray(a, dtype=np.float32).astype(BF))
    x_h = bf(inputs["x_h"]).reshape(B, HIGH, N)
    x_l = bf(inputs["x_l"]).reshape(B, LOW, N)
    theta_w = f(inputs["theta_w"])
    phi_w = f(inputs["phi_w"])
    g_w = f(inputs["g_w"])
    w_w = f(inputs["w_w"])

    thw_h = theta_w.T.reshape(KH, P, LOW).transpose(1, 0, 2).astype(BF)
    phw_h = (phi_w.T / np.float32(LOW)).reshape(KL, P, LOW).transpose(1, 0, 2).astype(BF)
    gw_h = g_w.T.reshape(KL, P, LOW).transpose(1, 0, 2).astype(BF)
    s = f(inputs["bn_gamma"]) / np.sqrt(f(inputs["bn_var"]) + np.float32(BN_EPS))
    # BN scale folded into the w conv weights; only the shift remains on-device
    ww_h = (w_w * s[:, None]).astype(np.float32).T.reshape(KL, P, HIGH) \
        .transpose(1, 0, 2).astype(BF)

    thpb_h = np.concatenate(
        [f(inputs["theta_b"]), f(inputs["phi_b"]) / np.float32(LOW)]
    ).reshape(1, 2 * LOW).astype(BF)
    gb_h = np.ascontiguousarray(f(inputs["g_b"]).reshape(KL, P).T)
    t = (f(inputs["w_b"]) - f(inputs["bn_mean"])) * s + f(inputs["bn_beta"])
    bnt_h = np.ascontiguousarray(t.astype(np.float32).reshape(KH, P).T)
    gbnt_h = np.ascontiguousarray(np.concatenate([gb_h, bnt_h], axis=1))

    shared = dict(
        thw=np.ascontiguousarray(thw_h),
        phw=np.ascontiguousarray(phw_h),
        gw=np.ascontiguousarray(gw_h),
        ww=np.ascontiguousarray(ww_h),
        thpb=thpb_h,
        gbnt=gbnt_h,
    )
    maps = []
    for c in range(NCORES):
        m = dict(shared)
        m["x_h"] = np.ascontiguousarray(x_h[c * BPC : (c + 1) * BPC])
        m["x_l"] = np.ascontiguousarray(x_l[c * BPC : (c + 1) * BPC])
        maps.append(m)
    return maps


def _run(inputs: dict, **kwargs):
    from concourse.bass_utils import run_bass_kernel_spmd

    nc = _get_module()
    in_maps = _prep_maps(inputs)
    res = run_bass_kernel_spmd(nc, in_maps, core_ids=list(range(NCORES)), **kwargs)
    parts = [np.asarray(r["out"], dtype=np.float32) for r in res.results]
    full = np.concatenate(parts, axis=0).reshape(B, HIGH, H, W)
    return full, res


def kernel(**inputs) -> np.ndarray:
    full, _ = _run(inputs)
    return full


# revision 13
# speedup vs baseline: 1.0252x; 1.0040x over previous
"""Trainium2 Bass kernel for the non-local-block module (nn_CNL_747324309589).

Sharding: data-parallel over batch — 16 batches across 8 NeuronCores, 2 per
core, no collectives.  Per batch (dims: HIGH=2048, LOW=512, N=H*W=1152):

    theta_xT[n,c] = sum_h xh[h,n]·thwT[h,c] + thb[c]      (x_h chunks = lhsT)
    phi_xT [n,d]  = sum_l xl[l,n]·phwT[l,d] + phb[d]      (phw,phb pre-scaled by 1/512)
    g_x    [d,n]  = sum_l gwT[l,d]·xl[l,n]  + gb[d]
    attT   [d,c]  = sum_n phi_xT[n,d]·theta_xT[n,c]       (= energy^T/512)
    y      [c,n]  = sum_d attT[d,c]·g_x[d,n]
    w_y    [o,n]  = sum_c wwT[c,o]·y[c,n]                 (BN scale pre-folded into ww)
    out    [o,n]  = w_y + bnt[o] + xh[o,n]                (shift + residual in one DVE op)

All matmul operands are bf16 (same 1 row/cycle PE rate as float32r in the
cost model, half the DMA bytes and SBUF footprint), accumulating fp32 in
PSUM; the output is DMA'd out as bf16 and widened to fp32 on the host.
Weights are loaded once (not per batch).  theta accumulates k-major across 8
PSUM banks so each (thw quarter, x_h chunk) pair is consumed as it lands;
phase order A2(phi), A1(theta), A3(g) puts g between theta's drain burst and
B1.  Batch b+1's x_l / x_h prefetch DMAs issue from the otherwise idle ACT
queue inside batch b's C loop.
"""

import numpy as np

import concourse.bass as bass
import concourse.bacc as bacc
import concourse.mybir as mybir
import concourse.tile as tile
from concourse.bass import ts

B, HIGH, LOW, H, W = 16, 2048, 512, 48, 24
N = H * W            # 1152
NCORES = 8
BPC = B // NCORES    # 2 batches per core
P = 128
KH = HIGH // P       # 16
KL = LOW // P        # 4
MN = N // P          # 9
NSPLIT = 3
NW = N // NSPLIT     # 384
BN_EPS = 1e-5

F32 = mybir.dt.float32
BF16 = mybir.dt.bfloat16
ADD = mybir.AluOpType.add
AF = mybir.ActivationFunctionType


def _build_module() -> bass.Bass:
    nc = bacc.Bacc()
    x_h = nc.dram_tensor("x_h", [BPC, HIGH, N], BF16, kind="ExternalInput")
    x_l = nc.dram_tensor("x_l", [BPC, LOW, N], BF16, kind="ExternalInput")
    thw = nc.dram_tensor("thw", [P, KH, LOW], BF16, kind="ExternalInput")
    phw = nc.dram_tensor("phw", [P, KL, LOW], BF16, kind="ExternalInput")
    gw = nc.dram_tensor("gw", [P, KL, LOW], BF16, kind="ExternalInput")
    ww = nc.dram_tensor("ww", [P, KL, HIGH], BF16, kind="ExternalInput")
    thpb = nc.dram_tensor("thpb", [1, 2 * LOW], BF16, kind="ExternalInput")
    gbnt = nc.dram_tensor("gbnt", [P, KL + KH], F32, kind="ExternalInput")
    out = nc.dram_tensor("out", [BPC, HIGH, N], BF16, kind="ExternalOutput")

    with tile.TileContext(nc) as tc:
        with (
            tc.tile_pool(name="consts", bufs=1) as cpool,
            tc.tile_pool(name="xh", bufs=KH) as xhpool,
            tc.tile_pool(name="xl", bufs=1) as xlpool,
            tc.tile_pool(name="att", bufs=1) as attpool,
            tc.tile_pool(name="mid", bufs=1) as midpool,
            tc.tile_pool(name="stg", bufs=3) as stgpool,
            tc.tile_pool(name="psum", bufs=8, space="PSUM") as pspool,
        ):
            # PE warmup: the p-state ramp (0.65/1.2 GHz for the first ~3us of
            # PE activity) burns on throwaway matmuls while the first DMAs
            # land, so real matmuls start at the full 2.4 GHz clock
            wu = cpool.tile([P, 64], BF16, tag="wu")
            nc.gpsimd.memset(wu[:], 0.0)
            wps = pspool.tile([P, 512], F32, tag="ps", name="wps")
            for i in range(62):
                nc.tensor.matmul(
                    wps[:64, :64], wu[:], wu[:], start=True, stop=True
                )

            # A2's inputs go first so phi can start ASAP.  Each dma_start
            # costs ~1.2us serialized on its issuing queue, so the three
            # first-dependency transfers are spread across SP, Pool (SWDGE)
            # and ACT so their issue latencies overlap.
            phw_sb = cpool.tile([P, KL, LOW], BF16, tag="phw")
            nc.sync.dma_start(phw_sb[:, :2], phw[:, :2])
            nc.gpsimd.dma_start(phw_sb[:, 2:], phw[:, 2:])
            xl0_sb = xlpool.tile([P, KL, N], BF16, tag="xl")
            xl0_r = x_l[0].rearrange("(ko p) n -> p ko n", p=P)
            # m=0 sub-chunk first so A2's first group starts on ~100KB
            nc.scalar.dma_start(xl0_sb[:, :, :P], xl0_r[:, :, :P])
            nc.scalar.dma_start(xl0_sb[:, :, P:NW], xl0_r[:, :, P:NW])
            for nn in range(1, NSPLIT):
                nc.scalar.dma_start(
                    xl0_sb[:, :, ts(nn, NW)], xl0_r[:, :, ts(nn, NW)]
                )
            thpb_sb = cpool.tile([P, 2 * LOW], BF16, tag="thpb")
            nc.sync.dma_start(thpb_sb[:], thpb[:].to_broadcast((P, 2 * LOW)))
            thb_sb = thpb_sb[:, :LOW]
            phb_sb = thpb_sb[:, LOW:]
            gbnt_sb = cpool.tile([P, KL + KH], F32, tag="gbnt")
            nc.sync.dma_start(gbnt_sb[:], gbnt[:])
            gb_sb = gbnt_sb[:, :KL]
            bnt_sb = gbnt_sb[:, KL:]
            # theta weights + batch-0 x_h chunks, interleaved in k-major
            # consumption order (A1 uses thw[k], xh[k] at k-step k)
            thw_sb = cpool.tile([P, KH, LOW], BF16, tag="thw")
            xh_t: list = [None] * KH
            for q in range(4):
                nc.sync.dma_start(
                    thw_sb[:, ts(q, KH // 4), :], thw[:, ts(q, KH // 4), :]
                )
                for k in range(q * 4, q * 4 + 4):
                    t_ = xhpool.tile([P, N], BF16, tag="xh")
                    nc.sync.dma_start(t_[:], x_h[0, ts(k, P), :])
                    xh_t[k] = t_
            gw_sb = cpool.tile([P, KL, LOW], BF16, tag="gw")
            nc.scalar.dma_start(gw_sb[:], gw[:])
            ww_sb = cpool.tile([P, KL, HIGH], BF16, tag="ww")
            for k in range(KL):
                nc.sync.dma_start(ww_sb[:, k], ww[:, k])

            for b in range(BPC):
                if b > 0:
                    xl_sb = xl_next
                    xh_t = xh_next

                else:
                    xl_sb = xl0_sb

                # phi_xT [n, d] (phase A2)
                ph_sb = midpool.tile([P, MN, LOW], BF16, tag="ph")
                for m in range(MN):
                    ps = pspool.tile([P, 512], F32, tag="ps")
                    for k in range(KL):
                        nc.tensor.matmul(
                            ps[:],
                            xl_sb[:, k, ts(m, P)],
                            phw_sb[:, k, :],
                            start=(k == 0),
                            stop=(k == KL - 1),
                        )
                    nc.vector.tensor_tensor(ph_sb[:, m, :], ps[:], phb_sb[:], ADD)

                # theta_xT [n, c] (phase A1) — k-major over 8 PSUM banks so
                # chunk k is consumed right after it lands; m=8 runs m-major
                th_sb = midpool.tile([P, MN, LOW], BF16, tag="th")
                ps_a1 = [
                    pspool.tile([P, 512], F32, tag="ps", name=f"ps_a1_{m}")
                    for m in range(8)
                ]
                for k in range(KH):
                    for m in range(8):
                        nc.tensor.matmul(
                            ps_a1[m][:],
                            xh_t[k][:, ts(m, P)],
                            thw_sb[:, k, :],
                            start=(k == 0),
                            stop=(k == KH - 1),
                        )
                for m in range(8):
                    nc.vector.tensor_tensor(th_sb[:, m, :], ps_a1[m][:], thb_sb[:], ADD)
                ps = pspool.tile([P, 512], F32, tag="ps")
                for k in range(KH):
                    nc.tensor.matmul(
                        ps[:],
                        xh_t[k][:, ts(8, P)],
                        thw_sb[:, k, :],
                        start=(k == 0),
                        stop=(k == KH - 1),
                    )
                nc.vector.tensor_tensor(th_sb[:, 8, :], ps[:], thb_sb[:], ADD)

                # g_x [d, n] (phase A3) — sits between theta's drain burst
                # and B1 so the th drains overlap PE work
                g_sb = midpool.tile([P, KL, N], BF16, tag="g")
                for md in range(KL):
                    for nn in range(NSPLIT):
                        ps = pspool.tile([P, 512], F32, tag="ps")
                        for k in range(KL):
                            nc.tensor.matmul(
                                ps[:, :NW],
                                gw_sb[:, k, ts(md, P)],
                                xl_sb[:, k, ts(nn, NW)],
                                start=(k == 0),
                                stop=(k == KL - 1),
                            )
                        nc.scalar.activation(
                            g_sb[:, md, ts(nn, NW)],
                            ps[:, :NW],
                            AF.Identity,
                            bias=gb_sb[:, md : md + 1],
                        )

                # batch b+1 x_l prefetch: WAR on this batch's A2/A3 reads,
                # issued from the ACT queue (behind A3's drains)
                if b + 1 < BPC:
                    xl_next = xlpool.tile([P, KL, N], BF16, tag="xl")
                    xl1_r = x_l[b + 1].rearrange("(ko p) n -> p ko n", p=P)
                    for nn in range(NSPLIT):
                        nc.scalar.dma_start(
                            xl_next[:, :, ts(nn, NW)], xl1_r[:, :, ts(nn, NW)]
                        )

                # attT [d, c] = energy^T/512 (phase B1)
                att_sb = attpool.tile([P, KL, LOW], BF16, tag="att")
                for md in range(KL):
                    ps = pspool.tile([P, 512], F32, tag="ps")
                    for k in range(MN):
                        nc.tensor.matmul(
                            ps[:],
                            ph_sb[:, k, ts(md, P)],
                            th_sb[:, k, :],
                            start=(k == 0),
                            stop=(k == MN - 1),
                        )
                    nc.scalar.activation(att_sb[:, md, :], ps[:], AF.Identity)

                # y [c, n] (phase B2); y shares the theta_xT slot
                y_sb = midpool.tile([P, KL, N], BF16, tag="th")
                for mc in range(KL):
                    for nn in range(NSPLIT):
                        ps = pspool.tile([P, 512], F32, tag="ps")
                        for k in range(KL):
                            nc.tensor.matmul(
                                ps[:, :NW],
                                att_sb[:, k, ts(mc, P)],
                                g_sb[:, k, ts(nn, NW)],
                                start=(k == 0),
                                stop=(k == KL - 1),
                            )
                        nc.scalar.activation(
                            y_sb[:, mc, ts(nn, NW)], ps[:, :NW], AF.Identity
                        )

                # w_y + BN + residual (phase C); output staged per mo stripe
                # and written as one DMA; batch b+1 x_h chunk prefetch issues
                # from ACT right after chunk mo's last read
                for mo in range(KH):
                    xt = xh_t[mo]
                    stg = stgpool.tile([P, N], BF16, tag="stg")
                    last = b == BPC - 1 and mo == KH - 1
                    for nn in range(NSPLIT):
                        ps = pspool.tile([P, 512], F32, tag="ps")
                        for k in range(KL):
                            nc.tensor.matmul(
                                ps[:, :NW],
                                ww_sb[:, k, ts(mo, P)],
                                y_sb[:, k, ts(nn, NW)],
                                start=(k == 0),
                                stop=(k == KL - 1),
                            )
                        nc.vector.scalar_tensor_tensor(
                            stg[:, ts(nn, NW)],
                            ps[:, :NW],
                            bnt_sb[:, mo : mo + 1],
                            xt[:, ts(nn, NW)],
                            ADD,
                            ADD,
                        )
                        # the very last stripe writes out in two DMAs so the
                        # final transfer after the last matmul is small
                        if last and nn == NSPLIT - 2:
                            nc.sync.dma_start(
                                out[b, ts(mo, P), : 2 * NW], stg[:, : 2 * NW]
                            )
                        elif last and nn == NSPLIT - 1:
                            nc.sync.dma_start(
                                out[b, ts(mo, P), 2 * NW :], stg[:, 2 * NW :]
                            )
                    if not last:
                        nc.sync.dma_start(out[b, ts(mo, P), :], stg[:])
                    if b + 1 < BPC:
                        if mo == 0:
                            xh_next = [None] * KH
                        t_ = xhpool.tile([P, N], BF16, tag="xh")
                        nc.scalar.dma_start(t_[:], x_h[b + 1, ts(mo, P), :])
                        xh_next[mo] = t_
    nc.compile()
    return nc


_CACHE: dict = {}


def _get_module() -> bass.Bass:
    if "nc" not in _CACHE:
        _CACHE["nc"] = _build_module()
    return _CACHE["nc"]


def _prep_maps(inputs: dict) -> list[dict]:
    import ml_dtypes

    BF = ml_dtypes.bfloat16
    f = lambda a: np.ascontiguousarray(np.asarray(a, dtype=np.float32))
    bf = lambda a: np.ascontiguousarray(np.asarray(a, dtype=np.float32).astype(BF))
    x_h = bf(inputs["x_h"]).reshape(B, HIGH, N)
    x_l = bf(inputs["x_l"]).reshape(B, LOW, N)
    theta_w = f(inputs["theta_w"])
    phi_w = f(inputs["phi_w"])
    g_w = f(inputs["g_w"])
    w_w = f(inputs["w_w"])

    thw_h = theta_w.T.reshape(KH, P, LOW).transpose(1, 0, 2).astype(BF)
    phw_h = (phi_w.T / np.float32(LOW)).reshape(KL, P, LOW).transpose(1, 0, 2).astype(BF)
    gw_h = g_w.T.reshape(KL, P, LOW).transpose(1, 0, 2).astype(BF)
    s = f(inputs["bn_gamma"]) / np.sqrt(f(inputs["bn_var"]) + np.float32(BN_EPS))
    # BN scale folded into the w conv weights; only the shift remains on-device
    ww_h = (w_w * s[:, None]).astype(np.float32).T.reshape(KL, P, HIGH) \
        .transpose(1, 0, 2).astype(BF)

    thpb_h = np.concatenate(
        [f(inputs["theta_b"]), f(inputs["phi_b"]) / np.float32(LOW)]
    ).reshape(1, 2 * LOW).astype(BF)
    gb_h = np.ascontiguousarray(f(inputs["g_b"]).reshape(KL, P).T)
    t = (f(inputs["w_b"]) - f(inputs["bn_mean"])) * s + f(inputs["bn_beta"])
    bnt_h = np.ascontiguousarray(t.astype(np.float32).reshape(KH, P).T)
    gbnt_h = np.ascontiguousarray(np.concatenate([gb_h, bnt_h], axis=1))

    shared = dict(
        thw=np.ascontiguousarray(thw_h),
        phw=np.ascontiguousarray(phw_h),
        gw=np.ascontiguousarray(gw_h),
        ww=np.ascontiguousarray(ww_h),
        thpb=thpb_h,
        gbnt=gbnt_h,
    )
    maps = []
    for c in range(NCORES):
        m = dict(shared)
        m["x_h"] = np.ascontiguousarray(x_h[c * BPC : (c + 1) * BPC])
        m["x_l"] = np.ascontiguousarray(x_l[c * BPC : (c + 1) * BPC])
        maps.append(m)
    return maps


def _run(inputs: dict, **kwargs):
    from concourse.bass_utils import run_bass_kernel_spmd

    nc = _get_module()
    in_maps = _prep_maps(inputs)
    res = run_bass_kernel_spmd(nc, in_maps, core_ids=list(range(NCORES)), **kwargs)
    parts = [np.asarray(r["out"], dtype=np.float32) for r in res.results]
    full = np.concatenate(parts, axis=0).reshape(B, HIGH, H, W)
    return full, res


def kernel(**inputs) -> np.ndarray:
    full, _ = _run(inputs)
    return full
